# revision 2
# baseline (speedup 1.0000x reference)
"""AuthorGroupAttention Trainium2 kernel, v4.

Data-parallel over batch: 8 samples -> 8 NeuronCores, one sample per core.
Per-sample routing resolved on host (combined [gen|rdr] projection weights).

Main path is bf16 (PE floor); the reader path (probability weight
0.05/0.5 = 10% -> 10x noise attenuation) runs in fp8e4m3 with DoubleRow
matmuls at 0.5 cyc/row:
  - rdr scores: stationary (kr8, zero-plane) [64,2,128], moving qr8
    duplicated stride-0 [64,2,512].
  - rdr attention: s-tile-paired DR: stationary v8 pairs, moving exr8 pairs.
Z rides in the attention stationary as indicator columns (value 1.0); tail:
reciprocal off the PSUM Z row -> DMA shift to partition 0 -> partition
broadcast -> fused STT normalize (x W, x 1/Z) -> combine add. The v-bias
folds into the output bias (probs rows sum to 0.5 exactly).

Schedule: software-pipelined. V-projection and head-0 scores interleave at
the start; for h>=1, scores(h) are emitted before attention(h-1) so the
scalar engine never stalls at head boundaries; projection of head h+1 is
pumped 4 steps per s-tile into the scores loop of head h.
"""

import os
import sys

for _p in ("/opt/trn_rl_repo",):
    if os.path.isdir(_p) and _p not in sys.path:
        sys.path.insert(0, _p)

import numpy as np
import ml_dtypes

import concourse.bass as bass
import concourse.mybir as mybir
from concourse import bacc
from concourse.tile import TileContext
from concourse.bass_utils import run_bass_kernel_spmd

B, T, E, H, G = 8, 1024, 1024, 16, 4
D = E // H  # 64
HP = H // 2
SCALING = float(D) ** -0.5
W_G = 0.9 / 2.0
W_R = 0.1 / 2.0
SW = 16.0

F32 = mybir.dt.float32
BF16 = mybir.dt.bfloat16
F8 = mybir.dt.float8e4
NPBF = ml_dtypes.bfloat16
NPF8 = ml_dtypes.float8_e4m3
DRM = mybir.MatmulPerfMode.DoubleRow
ADD = mybir.AluOpType.add
MULT = mybir.AluOpType.mult
EXP = mybir.ActivationFunctionType.Exp

EO = E // 128
SO = T // 128
OO = E // 128
VB = 192


def build_nc():
    nc = bacc.Bacc(name="author_group_attention_v4")

    hsT = nc.dram_tensor("hsT", [E, T], BF16, kind="ExternalInput")
    wq = nc.dram_tensor("wq", [E, H, 128], BF16, kind="ExternalInput")
    wk = nc.dram_tensor("wk", [E, H, 128], BF16, kind="ExternalInput")
    wv = nc.dram_tensor("wv", [E, E], BF16, kind="ExternalInput")
    wo = nc.dram_tensor("wo", [E, E], BF16, kind="ExternalInput")
    bqk = nc.dram_tensor("bqk", [128, 2 * H], F32, kind="ExternalInput")
    bo = nc.dram_tensor("bo", [128, OO], F32, kind="ExternalInput")
    outT = nc.dram_tensor("outT", [E, T], F32, kind="ExternalOutput")

    with TileContext(nc) as tc:
        from contextlib import ExitStack

        with ExitStack() as stack:
            const = stack.enter_context(tc.tile_pool(name="const", bufs=1))
            mid_stack = ExitStack()
            ppsum = mid_stack.enter_context(
                tc.tile_pool(name="ppsum", bufs=1, space="PSUM")
            )
            gpsum = mid_stack.enter_context(
                tc.tile_pool(name="gpsum", bufs=1, space="PSUM")
            )
            rpsum = mid_stack.enter_context(
                tc.tile_pool(name="rpsum", bufs=1, space="PSUM")
            )
            wpool = stack.enter_context(tc.tile_pool(name="wqk", bufs=3))
            qkpool = stack.enter_context(tc.tile_pool(name="qk", bufs=2))
            exgp = stack.enter_context(tc.tile_pool(name="exg", bufs=18))
            exrp = stack.enter_context(tc.tile_pool(name="exr", bufs=10))
            zp = stack.enter_context(tc.tile_pool(name="z", bufs=2))
            bcp = stack.enter_context(tc.tile_pool(name="bc", bufs=2))
            rawp = stack.enter_context(tc.tile_pool(name="raw", bufs=2))


            hsT_sb = const.tile([128, EO, T], BF16, tag="hsT")
            hsT_r = hsT.rearrange("(eo ep) t -> ep eo t", ep=128)
            v_sb = const.tile([128, SO, HP, VB], BF16, tag="v")
            v8_sb = const.tile([128, SO, HP, VB], F8, tag="v8")
            U16 = mybir.dt.uint16
            nc.vector.memset(v_sb[:, :, :, D : 2 * D].bitcast(mybir.dt.uint32), 0)
            nc.vector.memset(v_sb[:, :, :, D].bitcast(U16), 0x3F80)
            nc.vector.memset(v_sb[:, :, :, 96].bitcast(U16), 0x3F80)
            comb = const.tile([128, EO, T], BF16, tag="comb")
            bqk_sb = const.tile([128, 2 * H], F32, tag="bqk")
            bo_sb = const.tile([128, OO], F32, tag="bo")
            qr8_t = const.tile([128, 2, T], F8, tag="qr8")
            kr8_t = const.tile([128, 2, 2, T], F8, tag="kr8")
            nc.vector.memset(kr8_t[:, :, 1, :].bitcast(mybir.dt.uint32), 0)

            # head-0 weights + bias first on the SP queue; hsT split across
            # the SP and ACT queues (Pool's software DGE costs ~1us/DMA on
            # the Pool engine itself -- reserve it for non-urgent weights)
            nc.sync.dma_start(bqk_sb[:], bqk[:])

            def emit_hsT():
                for eo in range(4):
                    nc.sync.dma_start(hsT_sb[:, eo], hsT_r[:, eo])
                for eo in range(4, EO):
                    nc.scalar.dma_start(hsT_sb[:, eo], hsT_r[:, eo])

            def proj_steps(h, which):
                """Emission steps for combined Q or K proj of head h."""
                wt = wpool.tile([128, EO, 128], BF16, tag="w")
                srcw = wq if which == "q" else wk
                wq_queue = nc.scalar if h == 0 else nc.sync
                wq_queue.dma_start(
                    wt[:], srcw[:, h, :].rearrange("(eo ep) m -> ep eo m", ep=128)
                )
                dst = qkpool.tile([64, T], BF16, tag=which)
                hb = h % 2
                bias_col = 2 * h if which == "q" else 2 * h + 1
                state = {}

                pool_, ptag = (
                    (gpsum, "sg") if (h == 0 and which == "k") else (ppsum, "proj")
                )

                def mk_mm(nh, eo):
                    def step():
                        if nh == 0 and eo == 0:
                            state[0] = pool_.tile([128, T], F32, tag=ptag,
                                                  name=f"p{which}{h}")
                        nc.tensor.matmul(
                            state[0][:, nh * 512 : (nh + 1) * 512],
                            wt[:, eo, :],
                            hsT_sb[:, eo, nh * 512 : (nh + 1) * 512],
                            start=(eo == 0),
                            stop=(eo == EO - 1),
                        )
                        if eo == EO - 1:
                            sl = slice(nh * 512, (nh + 1) * 512)
                            nc.vector.tensor_scalar_add(
                                dst[:, sl],
                                state[0][0:64, sl],
                                bqk_sb[0:64, bias_col : bias_col + 1],
                            )
                            if which == "q":
                                r8dst = qr8_t[64:128, hb, sl]
                            else:
                                r8dst = kr8_t[64:128, hb, 0, sl]
                            # rdr bias rows arrive pre-scaled by SW from the
                            # host, so the drain is in*SW + bias16. Heads 0/1
                            # use the scalar engine (idle during the start).
                            if h <= 1:
                                nc.scalar.activation(
                                    r8dst,
                                    state[0][64:128, sl],
                                    mybir.ActivationFunctionType.Identity,
                                    bias=bqk_sb[64:128, bias_col : bias_col + 1],
                                    scale=SW,
                                )
                            else:
                                nc.vector.tensor_scalar(
                                    r8dst,
                                    state[0][64:128, sl],
                                    SW,
                                    bqk_sb[64:128, bias_col : bias_col + 1],
                                    MULT,
                                    ADD,
                                )
                    return step

                return dst, [mk_mm(nh, eo) for nh in range(2) for eo in range(EO)]

            def scores_step(h, s, Qh, Kh, exg_tiles, exr_tiles):
                hb = h % 2
                ssl = slice(s * 128, (s + 1) * 128)
                # rdr first: its psum ring frees one ACT op earlier, so the
                # scalar engine never idles across head boundaries
                psr = rpsum.tile([128, T], F32, tag="sr")
                qr_mv = qr8_t[64:128, hb, :]
                for nh in range(2):
                    tsl = slice(nh * 512, (nh + 1) * 512)
                    nc.tensor.matmul(
                        psr[:, tsl],
                        kr8_t[64:128, hb, :, ssl],
                        qr_mv[:, None, tsl].broadcast_to((64, 2, 512)),
                        start=True,
                        stop=True,
                        perf_mode=DRM,
                    )
                psg = gpsum.tile([128, T], F32, tag="sg")
                for nh in range(2):
                    nc.tensor.matmul(
                        psg[:, nh * 512 : (nh + 1) * 512],
                        Kh[:, ssl],
                        Qh[:, nh * 512 : (nh + 1) * 512],
                        start=True,
                        stop=True,
                    )
                if s % 2 == 0:
                    exr_tiles.append(exrp.tile([128, 2, T], F8, tag="exr", name=f"exr{h}_{s}"))
                nc.scalar.activation(
                    exr_tiles[s // 2][:, s % 2, :], psr[:], EXP,
                    scale=SCALING / (SW * SW),
                )
                exg = exgp.tile([128, T], BF16, tag="exg", name=f"exg{h}_{s}")
                nc.scalar.activation(exg[:], psg[:], EXP, scale=SCALING)
                exg_tiles.append(exg)

            def attn_steps(h, exg_tiles, exr_tiles, apsum):
                """Attention + normalize for head h as a list of emission
                steps (matmul chunks + tail), so it can be pumped into the
                next head's scores loop."""
                par_odd = h % 2
                abase = 64 * par_odd
                zrow = 64 if par_odd == 0 else 32
                voff = 64 * par_odd
                hp = h // 2
                steps = []
                state = {}

                def mk_mms(th, grp):
                    def step():
                        tsl = slice(th * 512, (th + 1) * 512)
                        if grp == 0:
                            state[th] = (
                                apsum.tile([128, 512], F32, tag="ag",
                                           name=f"pag{h}_{th}"),
                                apsum.tile([128, 512], F32, tag="ar",
                                           name=f"par{h}_{th}"),
                            )
                        pag, par_ = state[th]
                        if grp < 2:
                            for s in range(4 * grp, 4 * grp + 4):
                                nc.tensor.matmul(
                                    pag[:],
                                    v_sb[:, s, hp, voff : voff + 128],
                                    exg_tiles[s][:, tsl],
                                    start=(s == 0),
                                    stop=(s == SO - 1),
                                )
                        else:
                            for i in range(SO // 2):
                                nc.tensor.matmul(
                                    par_[:],
                                    v8_sb[:, 2 * i : 2 * i + 2, hp,
                                          voff : voff + 128],
                                    exr_tiles[i][:, :, tsl],
                                    start=(i == 0),
                                    stop=(i == SO // 2 - 1),
                                    perf_mode=DRM,
                                )
                    return step

                def mk_tail(th):
                    def step():
                        tsl = slice(th * 512, (th + 1) * 512)
                        pag, par_ = state[th]
                        zz = zp.tile([128, 2 * 512], F32, tag="zz",
                                     name=f"zz{h}_{th}")
                        zsl = slice(zrow, zrow + 1)
                        nc.vector.reciprocal(zz[zsl, 0:512], pag[zsl, :])
                        nc.vector.reciprocal(zz[zsl, 512:1024], par_[zsl, :])
                        nc.sync.dma_start(zz[0:1, 0:512], zz[zsl, 0:512])
                        nc.sync.dma_start(zz[0:1, 512:1024], zz[zsl, 512:1024])
                        bcg = bcp.tile([128, 512], F32, tag="bg",
                                       name=f"bg{h}_{th}")
                        bcr = bcp.tile([128, 512], F32, tag="br",
                                       name=f"br{h}_{th}")
                        nc.gpsimd.partition_broadcast(bcg[:], zz[0:1, 0:512])
                        nc.gpsimd.partition_broadcast(bcr[:], zz[0:1, 512:1024])
                        asl = slice(abase, abase + 64)
                        rawg = rawp.tile([128, 512], F32, tag="rg",
                                         name=f"rg{h}_{th}")
                        rawr = rawp.tile([128, 512], F32, tag="rr",
                                         name=f"rr{h}_{th}")
                        nc.vector.scalar_tensor_tensor(
                            rawg[asl, :], pag[asl, :], W_G, bcg[asl, :],
                            MULT, MULT,
                        )
                        nc.vector.scalar_tensor_tensor(
                            rawr[asl, :], par_[asl, :], W_R, bcr[asl, :],
                            MULT, MULT,
                        )
                        nc.vector.tensor_add(
                            comb[asl, hp, tsl], rawg[asl, :], rawr[asl, :]
                        )
                    return step

                for th in range(2):
                    steps += [mk_mms(th, 0), mk_mms(th, 1), mk_mms(th, 2),
                              mk_tail(th)]
                return steps

            # ---- proj of head 0, emitted directly (DMA-paced) --------------
            Qh, steps_q0 = proj_steps(0, "q")
            Kh, steps_k0 = proj_steps(0, "k")
            emit_hsT()
            for st in steps_q0 + steps_k0:
                st()

            # ---- start phase: V projection interleaved with head-0 scores --
            exg_prev = []
            exr_prev = []
            with tc.tile_pool(name="wvp", bufs=1) as wvp, tc.tile_pool(
                name="vpsum", bufs=1, space="PSUM"
            ) as vpsum:
                wv_sb = wvp.tile([128, EO, E], BF16, tag="wv")
                wv_r = wv.rearrange("(eo ep) o -> ep eo o", ep=128)
                # dummy Pool read of the last hsT chunk: holds the wv DMA
                # issues back so they don't interleave with hsT on the DMA
                # channel and delay projection 0
                wvgate = wvp.tile([1, 2], BF16, tag="wvgate")
                nc.gpsimd.tensor_scalar(
                    wvgate[0:1, :], hsT_sb[0:1, EO - 1, 0:2], 1.0, None, MULT
                )
                for eo in range(EO):
                    nc.gpsimd.dma_start(wv_sb[:, eo], wv_r[:, eo])
                nc.gpsimd.dma_start(bo_sb[:], bo[:])
                nextQ, steps_q1 = proj_steps(1, "q")
                nextK, steps_k1 = proj_steps(1, "k")
                pump = steps_q1 + steps_k1
                for so in range(SO):
                    scores_step(0, so, Qh, Kh, exg_prev, exr_prev)
                    pv = vpsum.tile([128, T], F32, tag="vproj", name=f"pv{so}")
                    for eo in range(EO):
                        for nh in range(2):
                            nc.tensor.matmul(
                                pv[:, nh * 512 : (nh + 1) * 512],
                                hsT_sb[:, eo, so * 128 : (so + 1) * 128],
                                wv_sb[:, eo, nh * 512 : (nh + 1) * 512],
                                start=(eo == 0),
                                stop=(eo == EO - 1),
                            )
                    for _ in range(4):
                        if pump:
                            pump.pop(0)()
                    pv4 = pv.rearrange("p (m two d) -> p m two d", two=2, d=D)
                    nc.vector.tensor_copy(v_sb[:, so, :, 0:D], pv4[:, :, 0, :])
                    nc.vector.tensor_copy(
                        v_sb[:, so, :, 128 : 128 + D], pv4[:, :, 1, :]
                    )
                    nc.gpsimd.tensor_scalar(
                        v8_sb[:, so], v_sb[:, so], 1.0, None, MULT
                    )
                while pump:
                    pump.pop(0)()
            Qh, Kh = nextQ, nextK

            tailp = stack.enter_context(tc.tile_pool(name="tail", bufs=8))
            outp2 = stack.enter_context(tc.tile_pool(name="opart", bufs=6))

            rings = [(ppsum, "proj"), (gpsum, "sg"), (rpsum, "sr")]
            NPRE = 6
            opart = {}
            po_pre = {}

            def pre_chunk(j, elo, ehi):
                def step():
                    if elo == 0:
                        pool_, ptag = rings[j % 3]
                        po_pre[j] = pool_.tile(
                            [128, T], F32, tag=ptag, name=f"poA{j}"
                        )
                    for nh in range(2):
                        for eo in range(elo, ehi):
                            nc.tensor.matmul(
                                po_pre[j][:, nh * 512 : (nh + 1) * 512],
                                wo_tiles[j][:, eo, :],
                                comb[:, eo, nh * 512 : (nh + 1) * 512],
                                start=(eo == 0),
                                stop=(eo == ehi - 1 and ehi == 7),
                            )
                    if ehi == 7:
                        op = outp2.tile([128, T], BF16, tag="opart",
                                        name=f"opart{j}")
                        opart[j] = op
                        nc.vector.tensor_copy(op[:], po_pre[j][:])
                return step

            # ppsum-ring js (0,3,6) can prework during head 15's scores
            # (no proj(16) uses that ring); gpsum/rpsum js go in the flush.
            fill_pp = []
            fill = [pre_chunk(j, elo, ehi) for j in range(NPRE)
                    for (elo, ehi) in ((0, 4), (4, 7))]

            # ---- heads: scores(h) first, attention(h-1) + proj(h+1) pumped --
            apsum = mid_stack.enter_context(
                tc.tile_pool(name="apsum", bufs=1, space="PSUM")
            )
            wo_tiles = []
            for h in range(1, H + 1):
                exg_cur = []
                exr_cur = []
                atn = attn_steps(h - 1, exg_prev, exr_prev, apsum)
                if h < H:
                    pump = []
                    if h + 1 < H:
                        nextQ, steps_q = proj_steps(h + 1, "q")
                        nextK, steps_k = proj_steps(h + 1, "k")
                        pump = steps_q + steps_k
                    if h == H - 3:
                        wo_r = wo.rearrange(
                            "(eo ep) (oo m) -> oo ep eo m", ep=128, m=128
                        )
                        for j in range(OO):
                            wt = tailp.tile([128, EO, 128], BF16, tag="wo",
                                            name=f"wo{j}")
                            nc.gpsimd.dma_start(wt[:], wo_r[j])
                            wo_tiles.append(wt)
                    for s in range(SO):
                        scores_step(h, s, Qh, Kh, exg_cur, exr_cur)
                        for _ in range(4):
                            if pump:
                                pump.pop(0)()
                            elif h == H - 1 and fill_pp and s >= 2:
                                fill_pp.pop(0)()
                        if atn:
                            atn.pop(0)()
                    while pump:
                        pump.pop(0)()
                if h == H:
                    fill = fill_pp + fill
                    while atn:
                        atn.pop(0)()
                        if fill:
                            fill.pop(0)()
                    while fill:
                        fill.pop(0)()
                else:
                    while atn:
                        atn.pop(0)()
                exg_prev, exr_prev = exg_cur, exr_cur
                if h < H - 1:
                    Qh, Kh = nextQ, nextK

            # ---------------- output projection -----------------------------
            with tc.tile_pool(name="outsb", bufs=2) as outp:
                for j in range(OO):
                    wt = wo_tiles[j]
                    pool_, ptag = [(ppsum, "proj"), (gpsum, "sg"),
                                   (rpsum, "sr")][j % 3]
                    po = pool_.tile([128, T], F32, tag=ptag, name=f"po{j}")
                    ot = outp.tile([128, T], F32, tag="ot", name=f"ot{j}")
                    pre = j < NPRE
                    for nh in range(2):
                        sl = slice(nh * 512, (nh + 1) * 512)
                        for eo in range(7 if pre else 0, EO):
                            nc.tensor.matmul(
                                po[:, sl],
                                wt[:, eo, :],
                                comb[:, eo, sl],
                                start=(eo == 7 if pre else eo == 0),
                                stop=(eo == EO - 1),
                            )
                        if pre:
                            nc.vector.scalar_tensor_tensor(
                                ot[:, sl], po[:, sl],
                                bo_sb[:, j : j + 1], opart[j][:, sl],
                                ADD, ADD,
                            )
                        else:
                            nc.vector.tensor_scalar_add(
                                ot[:, sl], po[:, sl], bo_sb[:, j : j + 1]
                            )
                        nc.sync.dma_start(
                            outT[j * 128 : (j + 1) * 128, sl], ot[:, sl]
                        )

            mid_stack.close()

    nc.finalize()
    return nc


_NC_CACHE = {}


def get_nc():
    if "nc" not in _NC_CACHE:
        _NC_CACHE["nc"] = build_nc()
    return _NC_CACHE["nc"]


def _host_prep(hidden_states, reader_token, Wq, bq, Wk, bk, Wv, bv, Wo, bo,
               RWq, Rbq, RWk, Rbk, RWv, Rbv):
    f = np.float32
    hs = np.asarray(hidden_states, f)
    tok = np.asarray(reader_token).astype(np.int64)
    WqT = np.ascontiguousarray(np.asarray(Wq, f).T)
    WkT = np.ascontiguousarray(np.asarray(Wk, f).T)
    WvT = np.ascontiguousarray(np.asarray(Wv, f).T).astype(NPBF)
    WoT = np.ascontiguousarray(np.asarray(Wo, f).T).astype(NPBF)
    RWqT = np.transpose(np.asarray(RWq, f), (0, 2, 1))
    RWkT = np.transpose(np.asarray(RWk, f), (0, 2, 1))
    bq = np.asarray(bq, f); bk = np.asarray(bk, f)
    bv = np.asarray(bv, f); bo_ = np.asarray(bo, f)
    Rbq = np.asarray(Rbq, f); Rbk = np.asarray(Rbk, f)

    bo_eff = bo_ + 0.5 * (np.asarray(Wo, f) @ bv)
    bo_t = np.ascontiguousarray(bo_eff.reshape(OO, 128).T)

    WqT_h = WqT.reshape(E, H, D)
    WkT_h = WkT.reshape(E, H, D)

    in_maps = []
    percore = {}
    for b in range(B):
        g = int(tok[b])
        if g not in percore:
            wqc = np.empty((E, H, 128), f)
            wqc[:, :, :D] = WqT_h
            wqc[:, :, D:] = RWqT[g].reshape(E, H, D)
            wkc = np.empty((E, H, 128), f)
            wkc[:, :, :D] = WkT_h
            wkc[:, :, D:] = RWkT[g].reshape(E, H, D)
            bqk_t = np.empty((128, 2 * H), f)
            bqk_t[:D, 0::2] = bq.reshape(H, D).T
            bqk_t[D:, 0::2] = SW * Rbq[g].reshape(H, D).T
            bqk_t[:D, 1::2] = bk.reshape(H, D).T
            bqk_t[D:, 1::2] = SW * Rbk[g].reshape(H, D).T
            percore[g] = (wqc.astype(NPBF), wkc.astype(NPBF), bqk_t)
        wqc, wkc, bqk_t = percore[g]
        in_maps.append(
            {
                "hsT": np.ascontiguousarray(hs[b].T).astype(NPBF),
                "wq": wqc,
                "wk": wkc,
                "wv": WvT,
                "wo": WoT,
                "bqk": bqk_t,
                "bo": bo_t,
            }
        )
    return in_maps


def kernel(**inputs) -> np.ndarray:
    in_maps = _host_prep(**inputs)
    nc = get_nc()
    res = run_bass_kernel_spmd(nc, in_maps, list(range(B)))
    out = np.stack([res.results[c]["outT"].T for c in range(B)], axis=0)
    return np.ascontiguousarray(out.astype(np.float32))


# revision 3
# speedup vs baseline: 1.0073x; 1.0073x over previous
"""AuthorGroupAttention Trainium2 kernel, v4.

Data-parallel over batch: 8 samples -> 8 NeuronCores, one sample per core.
Per-sample routing resolved on host (combined [gen|rdr] projection weights).

Main path is bf16 (PE floor); the reader path (probability weight
0.05/0.5 = 10% -> 10x noise attenuation) runs in fp8e4m3 with DoubleRow
matmuls at 0.5 cyc/row:
  - rdr scores: stationary (kr8, zero-plane) [64,2,128], moving qr8
    duplicated stride-0 [64,2,512].
  - rdr attention: s-tile-paired DR: stationary v8 pairs, moving exr8 pairs.
Z rides in the attention stationary as indicator columns (value 1.0); tail:
reciprocal off the PSUM Z row -> DMA shift to partition 0 -> partition
broadcast -> fused STT normalize (x W, x 1/Z) -> combine add. The v-bias
folds into the output bias (probs rows sum to 0.5 exactly).

Schedule: software-pipelined. V-projection and head-0 scores interleave at
the start; for h>=1, scores(h) are emitted before attention(h-1) so the
scalar engine never stalls at head boundaries; projection of head h+1 is
pumped 4 steps per s-tile into the scores loop of head h.
"""

import os
import sys

for _p in ("/opt/trn_rl_repo",):
    if os.path.isdir(_p) and _p not in sys.path:
        sys.path.insert(0, _p)

import numpy as np
import ml_dtypes

import concourse.bass as bass
import concourse.mybir as mybir
from concourse import bacc
from concourse.tile import TileContext
from concourse.bass_utils import run_bass_kernel_spmd

B, T, E, H, G = 8, 1024, 1024, 16, 4
D = E // H  # 64
HP = H // 2
SCALING = float(D) ** -0.5
W_G = 0.9 / 2.0
W_R = 0.1 / 2.0
SW = 16.0

F32 = mybir.dt.float32
BF16 = mybir.dt.bfloat16
F8 = mybir.dt.float8e4
NPBF = ml_dtypes.bfloat16
NPF8 = ml_dtypes.float8_e4m3
DRM = mybir.MatmulPerfMode.DoubleRow
ADD = mybir.AluOpType.add
MULT = mybir.AluOpType.mult
EXP = mybir.ActivationFunctionType.Exp

EO = E // 128
SO = T // 128
OO = E // 128
VB = 192


def build_nc():
    nc = bacc.Bacc(name="author_group_attention_v4")

    hsT = nc.dram_tensor("hsT", [E, T], BF16, kind="ExternalInput")
    wq = nc.dram_tensor("wq", [E, H, 128], BF16, kind="ExternalInput")
    wk = nc.dram_tensor("wk", [E, H, 128], BF16, kind="ExternalInput")
    wv = nc.dram_tensor("wv", [E, E], BF16, kind="ExternalInput")
    wo = nc.dram_tensor("wo", [E, E], BF16, kind="ExternalInput")
    bqk = nc.dram_tensor("bqk", [128, 2 * H], F32, kind="ExternalInput")
    bo = nc.dram_tensor("bo", [128, OO], F32, kind="ExternalInput")
    outT = nc.dram_tensor("outT", [E, T], F32, kind="ExternalOutput")

    with TileContext(nc) as tc:
        from contextlib import ExitStack

        with ExitStack() as stack:
            const = stack.enter_context(tc.tile_pool(name="const", bufs=1))
            mid_stack = ExitStack()
            ppsum = mid_stack.enter_context(
                tc.tile_pool(name="ppsum", bufs=1, space="PSUM")
            )
            gpsum = mid_stack.enter_context(
                tc.tile_pool(name="gpsum", bufs=1, space="PSUM")
            )
            rpsum = mid_stack.enter_context(
                tc.tile_pool(name="rpsum", bufs=1, space="PSUM")
            )
            wpool = stack.enter_context(tc.tile_pool(name="wqk", bufs=3))
            qkpool = stack.enter_context(tc.tile_pool(name="qk", bufs=2))
            exgp = stack.enter_context(tc.tile_pool(name="exg", bufs=18))
            exrp = stack.enter_context(tc.tile_pool(name="exr", bufs=10))
            zp = stack.enter_context(tc.tile_pool(name="z", bufs=2))
            bcp = stack.enter_context(tc.tile_pool(name="bc", bufs=2))
            rawp = stack.enter_context(tc.tile_pool(name="raw", bufs=2))


            hsT_sb = const.tile([128, EO, T], BF16, tag="hsT")
            hsT_r = hsT.rearrange("(eo ep) t -> ep eo t", ep=128)
            v_sb = const.tile([128, SO, HP, VB], BF16, tag="v")
            v8_sb = const.tile([128, SO, HP, VB], F8, tag="v8")
            U16 = mybir.dt.uint16
            nc.vector.memset(v_sb[:, :, :, D : 2 * D].bitcast(mybir.dt.uint32), 0)
            nc.vector.memset(v_sb[:, :, :, D].bitcast(U16), 0x3F80)
            nc.vector.memset(v_sb[:, :, :, 96].bitcast(U16), 0x3F80)
            comb = const.tile([128, EO, T], BF16, tag="comb")
            bqk_sb = const.tile([128, 2 * H], F32, tag="bqk")
            bo_sb = const.tile([128, OO], F32, tag="bo")
            qr8_t = const.tile([128, 2, T], F8, tag="qr8")
            kr8_t = const.tile([128, 2, 2, T], F8, tag="kr8")
            nc.vector.memset(kr8_t[:, :, 1, :].bitcast(mybir.dt.uint32), 0)

            # head-0 weights + bias first on the SP queue; hsT split across
            # the SP and ACT queues (Pool's software DGE costs ~1us/DMA on
            # the Pool engine itself -- reserve it for non-urgent weights)
            nc.sync.dma_start(bqk_sb[:], bqk[:])

            def emit_hsT():
                for eo in range(4):
                    nc.sync.dma_start(hsT_sb[:, eo], hsT_r[:, eo])
                for eo in range(4, EO):
                    nc.scalar.dma_start(hsT_sb[:, eo], hsT_r[:, eo])

            def proj_steps(h, which):
                """Emission steps for combined Q or K proj of head h."""
                wt = wpool.tile([128, EO, 128], BF16, tag="w")
                srcw = wq if which == "q" else wk
                wq_queue = nc.scalar if h == 0 else nc.gpsimd
                wq_queue.dma_start(
                    wt[:], srcw[:, h, :].rearrange("(eo ep) m -> ep eo m", ep=128)
                )
                dst = qkpool.tile([64, T], BF16, tag=which)
                hb = h % 2
                bias_col = 2 * h if which == "q" else 2 * h + 1
                state = {}

                pool_, ptag = (
                    (gpsum, "sg") if (h == 0 and which == "k") else (ppsum, "proj")
                )

                def mk_mm(nh, eo):
                    def step():
                        if nh == 0 and eo == 0:
                            state[0] = pool_.tile([128, T], F32, tag=ptag,
                                                  name=f"p{which}{h}")
                        nc.tensor.matmul(
                            state[0][:, nh * 512 : (nh + 1) * 512],
                            wt[:, eo, :],
                            hsT_sb[:, eo, nh * 512 : (nh + 1) * 512],
                            start=(eo == 0),
                            stop=(eo == EO - 1),
                        )
                        if eo == EO - 1:
                            sl = slice(nh * 512, (nh + 1) * 512)
                            nc.vector.tensor_scalar_add(
                                dst[:, sl],
                                state[0][0:64, sl],
                                bqk_sb[0:64, bias_col : bias_col + 1],
                            )
                            if which == "q":
                                r8dst = qr8_t[64:128, hb, sl]
                            else:
                                r8dst = kr8_t[64:128, hb, 0, sl]
                            # rdr bias rows arrive pre-scaled by SW from the
                            # host, so the drain is in*SW + bias16. Heads 0/1
                            # use the scalar engine (idle during the start).
                            if h <= 1:
                                nc.scalar.activation(
                                    r8dst,
                                    state[0][64:128, sl],
                                    mybir.ActivationFunctionType.Identity,
                                    bias=bqk_sb[64:128, bias_col : bias_col + 1],
                                    scale=SW,
                                )
                            else:
                                nc.vector.tensor_scalar(
                                    r8dst,
                                    state[0][64:128, sl],
                                    SW,
                                    bqk_sb[64:128, bias_col : bias_col + 1],
                                    MULT,
                                    ADD,
                                )
                    return step

                return dst, [mk_mm(nh, eo) for nh in range(2) for eo in range(EO)]

            def scores_step(h, s, Qh, Kh, exg_tiles, exr_tiles):
                hb = h % 2
                ssl = slice(s * 128, (s + 1) * 128)
                # rdr first: its psum ring frees one ACT op earlier, so the
                # scalar engine never idles across head boundaries
                psr = rpsum.tile([128, T], F32, tag="sr")
                qr_mv = qr8_t[64:128, hb, :]
                for nh in range(2):
                    tsl = slice(nh * 512, (nh + 1) * 512)
                    nc.tensor.matmul(
                        psr[:, tsl],
                        kr8_t[64:128, hb, :, ssl],
                        qr_mv[:, None, tsl].broadcast_to((64, 2, 512)),
                        start=True,
                        stop=True,
                        perf_mode=DRM,
                    )
                psg = gpsum.tile([128, T], F32, tag="sg")
                for nh in range(2):
                    nc.tensor.matmul(
                        psg[:, nh * 512 : (nh + 1) * 512],
                        Kh[:, ssl],
                        Qh[:, nh * 512 : (nh + 1) * 512],
                        start=True,
                        stop=True,
                    )
                if s % 2 == 0:
                    exr_tiles.append(exrp.tile([128, 2, T], F8, tag="exr", name=f"exr{h}_{s}"))
                nc.scalar.activation(
                    exr_tiles[s // 2][:, s % 2, :], psr[:], EXP,
                    scale=SCALING / (SW * SW),
                )
                exg = exgp.tile([128, T], BF16, tag="exg", name=f"exg{h}_{s}")
                nc.scalar.activation(exg[:], psg[:], EXP, scale=SCALING)
                exg_tiles.append(exg)

            def attn_steps(h, exg_tiles, exr_tiles, apsum):
                """Attention + normalize for head h as a list of emission
                steps (matmul chunks + tail), so it can be pumped into the
                next head's scores loop."""
                par_odd = h % 2
                abase = 64 * par_odd
                zrow = 64 if par_odd == 0 else 32
                voff = 64 * par_odd
                hp = h // 2
                steps = []
                state = {}

                def mk_mms(th, grp):
                    def step():
                        tsl = slice(th * 512, (th + 1) * 512)
                        if grp == 0:
                            state[th] = (
                                apsum.tile([128, 512], F32, tag="ag",
                                           name=f"pag{h}_{th}"),
                                apsum.tile([128, 512], F32, tag="ar",
                                           name=f"par{h}_{th}"),
                            )
                        pag, par_ = state[th]
                        if grp < 2:
                            for s in range(4 * grp, 4 * grp + 4):
                                nc.tensor.matmul(
                                    pag[:],
                                    v_sb[:, s, hp, voff : voff + 128],
                                    exg_tiles[s][:, tsl],
                                    start=(s == 0),
                                    stop=(s == SO - 1),
                                )
                        else:
                            for i in range(SO // 2):
                                nc.tensor.matmul(
                                    par_[:],
                                    v8_sb[:, 2 * i : 2 * i + 2, hp,
                                          voff : voff + 128],
                                    exr_tiles[i][:, :, tsl],
                                    start=(i == 0),
                                    stop=(i == SO // 2 - 1),
                                    perf_mode=DRM,
                                )
                    return step

                def mk_tail(th):
                    def step():
                        tsl = slice(th * 512, (th + 1) * 512)
                        pag, par_ = state[th]
                        zz = zp.tile([128, 2 * 512], F32, tag="zz",
                                     name=f"zz{h}_{th}")
                        zsl = slice(zrow, zrow + 1)
                        nc.vector.reciprocal(zz[zsl, 0:512], pag[zsl, :])
                        nc.vector.reciprocal(zz[zsl, 512:1024], par_[zsl, :])
                        nc.sync.dma_start(zz[0:1, 0:512], zz[zsl, 0:512])
                        nc.sync.dma_start(zz[0:1, 512:1024], zz[zsl, 512:1024])
                        bcg = bcp.tile([128, 512], F32, tag="bg",
                                       name=f"bg{h}_{th}")
                        bcr = bcp.tile([128, 512], F32, tag="br",
                                       name=f"br{h}_{th}")
                        nc.gpsimd.partition_broadcast(bcg[:], zz[0:1, 0:512])
                        nc.gpsimd.partition_broadcast(bcr[:], zz[0:1, 512:1024])
                        asl = slice(abase, abase + 64)
                        rawg = rawp.tile([128, 512], F32, tag="rg",
                                         name=f"rg{h}_{th}")
                        rawr = rawp.tile([128, 512], F32, tag="rr",
                                         name=f"rr{h}_{th}")
                        nc.vector.scalar_tensor_tensor(
                            rawg[asl, :], pag[asl, :], W_G, bcg[asl, :],
                            MULT, MULT,
                        )
                        nc.vector.scalar_tensor_tensor(
                            rawr[asl, :], par_[asl, :], W_R, bcr[asl, :],
                            MULT, MULT,
                        )
                        nc.vector.tensor_add(
                            comb[asl, hp, tsl], rawg[asl, :], rawr[asl, :]
                        )
                    return step

                for th in range(2):
                    steps += [mk_mms(th, 0), mk_mms(th, 1), mk_mms(th, 2),
                              mk_tail(th)]
                return steps

            # ---- proj of head 0, emitted directly (DMA-paced) --------------
            Qh, steps_q0 = proj_steps(0, "q")
            Kh, steps_k0 = proj_steps(0, "k")
            emit_hsT()
            for st in steps_q0 + steps_k0:
                st()

            # ---- start phase: V projection interleaved with head-0 scores --
            exg_prev = []
            exr_prev = []
            with tc.tile_pool(name="wvp", bufs=1) as wvp, tc.tile_pool(
                name="vpsum", bufs=1, space="PSUM"
            ) as vpsum:
                wv_sb = wvp.tile([128, EO, E], BF16, tag="wv")
                wv_r = wv.rearrange("(eo ep) o -> ep eo o", ep=128)
                # dummy Pool read of the last hsT chunk: holds the wv DMA
                # issues back so they don't interleave with hsT on the DMA
                # channel and delay projection 0
                wvgate = wvp.tile([1, 2], BF16, tag="wvgate")
                nc.gpsimd.tensor_scalar(
                    wvgate[0:1, :], hsT_sb[0:1, EO - 1, 0:2], 1.0, None, MULT
                )
                for eo in range(EO):
                    nc.gpsimd.dma_start(wv_sb[:, eo], wv_r[:, eo])
                nc.gpsimd.dma_start(bo_sb[:], bo[:])
                nextQ, steps_q1 = proj_steps(1, "q")
                nextK, steps_k1 = proj_steps(1, "k")
                pump = steps_q1 + steps_k1
                for so in range(SO):
                    scores_step(0, so, Qh, Kh, exg_prev, exr_prev)
                    pv = vpsum.tile([128, T], F32, tag="vproj", name=f"pv{so}")
                    for eo in range(EO):
                        for nh in range(2):
                            nc.tensor.matmul(
                                pv[:, nh * 512 : (nh + 1) * 512],
                                hsT_sb[:, eo, so * 128 : (so + 1) * 128],
                                wv_sb[:, eo, nh * 512 : (nh + 1) * 512],
                                start=(eo == 0),
                                stop=(eo == EO - 1),
                            )
                    for _ in range(4):
                        if pump:
                            pump.pop(0)()
                    pv4 = pv.rearrange("p (m two d) -> p m two d", two=2, d=D)
                    nc.vector.tensor_copy(v_sb[:, so, :, 0:D], pv4[:, :, 0, :])
                    nc.vector.tensor_copy(
                        v_sb[:, so, :, 128 : 128 + D], pv4[:, :, 1, :]
                    )
                    nc.gpsimd.tensor_scalar(
                        v8_sb[:, so], v_sb[:, so], 1.0, None, MULT
                    )
                while pump:
                    pump.pop(0)()
            Qh, Kh = nextQ, nextK

            tailp = stack.enter_context(tc.tile_pool(name="tail", bufs=8))
            outp2 = stack.enter_context(tc.tile_pool(name="opart", bufs=6))

            rings = [(ppsum, "proj"), (gpsum, "sg"), (rpsum, "sr")]
            NPRE = 6
            opart = {}
            po_pre = {}

            def pre_chunk(j, elo, ehi):
                def step():
                    if elo == 0:
                        pool_, ptag = rings[j % 3]
                        po_pre[j] = pool_.tile(
                            [128, T], F32, tag=ptag, name=f"poA{j}"
                        )
                    for nh in range(2):
                        for eo in range(elo, ehi):
                            nc.tensor.matmul(
                                po_pre[j][:, nh * 512 : (nh + 1) * 512],
                                wo_tiles[j][:, eo, :],
                                comb[:, eo, nh * 512 : (nh + 1) * 512],
                                start=(eo == 0),
                                stop=(eo == ehi - 1 and ehi == 7),
                            )
                    if ehi == 7:
                        op = outp2.tile([128, T], BF16, tag="opart",
                                        name=f"opart{j}")
                        opart[j] = op
                        nc.vector.tensor_copy(op[:], po_pre[j][:])
                return step

            # ppsum-ring js (0,3,6) can prework during head 15's scores
            # (no proj(16) uses that ring); gpsum/rpsum js go in the flush.
            fill_pp = []
            fill = [pre_chunk(j, elo, ehi) for j in range(NPRE)
                    for (elo, ehi) in ((0, 4), (4, 7))]

            # ---- heads: scores(h) first, attention(h-1) + proj(h+1) pumped --
            apsum = mid_stack.enter_context(
                tc.tile_pool(name="apsum", bufs=1, space="PSUM")
            )
            wo_tiles = []
            for h in range(1, H + 1):
                exg_cur = []
                exr_cur = []
                atn = attn_steps(h - 1, exg_prev, exr_prev, apsum)
                if h < H:
                    pump = []
                    if h + 1 < H:
                        nextQ, steps_q = proj_steps(h + 1, "q")
                        nextK, steps_k = proj_steps(h + 1, "k")
                        pump = steps_q + steps_k
                    if h == H - 3:
                        wo_r = wo.rearrange(
                            "(eo ep) (oo m) -> oo ep eo m", ep=128, m=128
                        )
                        for j in range(OO):
                            wt = tailp.tile([128, EO, 128], BF16, tag="wo",
                                            name=f"wo{j}")
                            nc.gpsimd.dma_start(wt[:], wo_r[j])
                            wo_tiles.append(wt)
                    for s in range(SO):
                        scores_step(h, s, Qh, Kh, exg_cur, exr_cur)
                        for _ in range(4):
                            if pump:
                                pump.pop(0)()
                            elif h == H - 1 and fill_pp and s >= 2:
                                fill_pp.pop(0)()
                        if atn:
                            atn.pop(0)()
                    while pump:
                        pump.pop(0)()
                if h == H:
                    fill = fill_pp + fill
                    while atn:
                        atn.pop(0)()
                        if fill:
                            fill.pop(0)()
                    while fill:
                        fill.pop(0)()
                else:
                    while atn:
                        atn.pop(0)()
                exg_prev, exr_prev = exg_cur, exr_cur
                if h < H - 1:
                    Qh, Kh = nextQ, nextK

            # ---------------- output projection -----------------------------
            with tc.tile_pool(name="outsb", bufs=2) as outp:
                for j in range(OO):
                    wt = wo_tiles[j]
                    pool_, ptag = [(ppsum, "proj"), (gpsum, "sg"),
                                   (rpsum, "sr")][j % 3]
                    po = pool_.tile([128, T], F32, tag=ptag, name=f"po{j}")
                    ot = outp.tile([128, T], F32, tag="ot", name=f"ot{j}")
                    pre = j < NPRE
                    for nh in range(2):
                        sl = slice(nh * 512, (nh + 1) * 512)
                        for eo in range(7 if pre else 0, EO):
                            nc.tensor.matmul(
                                po[:, sl],
                                wt[:, eo, :],
                                comb[:, eo, sl],
                                start=(eo == 7 if pre else eo == 0),
                                stop=(eo == EO - 1),
                            )
                        if pre:
                            nc.vector.scalar_tensor_tensor(
                                ot[:, sl], po[:, sl],
                                bo_sb[:, j : j + 1], opart[j][:, sl],
                                ADD, ADD,
                            )
                        else:
                            nc.vector.tensor_scalar_add(
                                ot[:, sl], po[:, sl], bo_sb[:, j : j + 1]
                            )
                        nc.sync.dma_start(
                            outT[j * 128 : (j + 1) * 128, sl], ot[:, sl]
                        )

            mid_stack.close()

    nc.finalize()
    return nc


_NC_CACHE = {}


def get_nc():
    if "nc" not in _NC_CACHE:
        _NC_CACHE["nc"] = build_nc()
    return _NC_CACHE["nc"]


def _host_prep(hidden_states, reader_token, Wq, bq, Wk, bk, Wv, bv, Wo, bo,
               RWq, Rbq, RWk, Rbk, RWv, Rbv):
    f = np.float32
    hs = np.asarray(hidden_states, f)
    tok = np.asarray(reader_token).astype(np.int64)
    WqT = np.ascontiguousarray(np.asarray(Wq, f).T)
    WkT = np.ascontiguousarray(np.asarray(Wk, f).T)
    WvT = np.ascontiguousarray(np.asarray(Wv, f).T).astype(NPBF)
    WoT = np.ascontiguousarray(np.asarray(Wo, f).T).astype(NPBF)
    RWqT = np.transpose(np.asarray(RWq, f), (0, 2, 1))
    RWkT = np.transpose(np.asarray(RWk, f), (0, 2, 1))
    bq = np.asarray(bq, f); bk = np.asarray(bk, f)
    bv = np.asarray(bv, f); bo_ = np.asarray(bo, f)
    Rbq = np.asarray(Rbq, f); Rbk = np.asarray(Rbk, f)

    bo_eff = bo_ + 0.5 * (np.asarray(Wo, f) @ bv)
    bo_t = np.ascontiguousarray(bo_eff.reshape(OO, 128).T)

    WqT_h = WqT.reshape(E, H, D)
    WkT_h = WkT.reshape(E, H, D)

    in_maps = []
    percore = {}
    for b in range(B):
        g = int(tok[b])
        if g not in percore:
            wqc = np.empty((E, H, 128), f)
            wqc[:, :, :D] = WqT_h
            wqc[:, :, D:] = RWqT[g].reshape(E, H, D)
            wkc = np.empty((E, H, 128), f)
            wkc[:, :, :D] = WkT_h
            wkc[:, :, D:] = RWkT[g].reshape(E, H, D)
            bqk_t = np.empty((128, 2 * H), f)
            bqk_t[:D, 0::2] = bq.reshape(H, D).T
            bqk_t[D:, 0::2] = SW * Rbq[g].reshape(H, D).T
            bqk_t[:D, 1::2] = bk.reshape(H, D).T
            bqk_t[D:, 1::2] = SW * Rbk[g].reshape(H, D).T
            percore[g] = (wqc.astype(NPBF), wkc.astype(NPBF), bqk_t)
        wqc, wkc, bqk_t = percore[g]
        in_maps.append(
            {
                "hsT": np.ascontiguousarray(hs[b].T).astype(NPBF),
                "wq": wqc,
                "wk": wkc,
                "wv": WvT,
                "wo": WoT,
                "bqk": bqk_t,
                "bo": bo_t,
            }
        )
    return in_maps


def kernel(**inputs) -> np.ndarray:
    in_maps = _host_prep(**inputs)
    nc = get_nc()
    res = run_bass_kernel_spmd(nc, in_maps, list(range(B)))
    out = np.stack([res.results[c]["outT"].T for c in range(B)], axis=0)
    return np.ascontiguousarray(out.astype(np.float32))


# revision 4
# speedup vs baseline: 1.0517x; 1.0441x over previous
"""AuthorGroupAttention Trainium2 kernel, v4.

Data-parallel over batch: 8 samples -> 8 NeuronCores, one sample per core.
Per-sample routing resolved on host (combined [gen|rdr] projection weights).

Main path is bf16 (PE floor); the reader path (probability weight
0.05/0.5 = 10% -> 10x noise attenuation) runs in fp8e4m3 with DoubleRow
matmuls at 0.5 cyc/row:
  - rdr scores: stationary (kr8, zero-plane) [64,2,128], moving qr8
    duplicated stride-0 [64,2,512].
  - rdr attention: s-tile-paired DR: stationary v8 pairs, moving exr8 pairs.
Z rides in the attention stationary as indicator columns (value 1.0); tail:
reciprocal off the PSUM Z row -> DMA shift to partition 0 -> partition
broadcast -> fused STT normalize (x W, x 1/Z) -> combine add. The v-bias
folds into the output bias (probs rows sum to 0.5 exactly).

Schedule: software-pipelined. V-projection and head-0 scores interleave at
the start; for h>=1, scores(h) are emitted before attention(h-1) so the
scalar engine never stalls at head boundaries; projection of head h+1 is
pumped 4 steps per s-tile into the scores loop of head h.
"""

import os
import sys

for _p in ("/opt/trn_rl_repo",):
    if os.path.isdir(_p) and _p not in sys.path:
        sys.path.insert(0, _p)

import numpy as np
import ml_dtypes

import concourse.bass as bass
import concourse.mybir as mybir
from concourse import bacc
from concourse.tile import TileContext
from concourse.bass_utils import run_bass_kernel_spmd

B, T, E, H, G = 8, 1024, 1024, 16, 4
D = E // H  # 64
HP = H // 2
SCALING = float(D) ** -0.5
W_G = 0.9 / 2.0
W_R = 0.1 / 2.0
SW = 16.0

F32 = mybir.dt.float32
BF16 = mybir.dt.bfloat16
F8 = mybir.dt.float8e4
NPBF = ml_dtypes.bfloat16
NPF8 = ml_dtypes.float8_e4m3
DRM = mybir.MatmulPerfMode.DoubleRow
ADD = mybir.AluOpType.add
MULT = mybir.AluOpType.mult
EXP = mybir.ActivationFunctionType.Exp

EO = E // 128
SO = T // 128
OO = E // 128
VB = 192


def build_nc():
    nc = bacc.Bacc(name="author_group_attention_v4")

    hsT = nc.dram_tensor("hsT", [E, T], BF16, kind="ExternalInput")
    wq = nc.dram_tensor("wq", [E, H, 128], BF16, kind="ExternalInput")
    wk = nc.dram_tensor("wk", [E, H, 128], BF16, kind="ExternalInput")
    wv = nc.dram_tensor("wv", [E, E], BF16, kind="ExternalInput")
    wo = nc.dram_tensor("wo", [E, E], BF16, kind="ExternalInput")
    bqk = nc.dram_tensor("bqk", [128, 2 * H], F32, kind="ExternalInput")
    bo = nc.dram_tensor("bo", [128, OO], F32, kind="ExternalInput")
    outT = nc.dram_tensor("outT", [E, T], F32, kind="ExternalOutput")

    with TileContext(nc) as tc:
        from contextlib import ExitStack

        with ExitStack() as stack:
            const = stack.enter_context(tc.tile_pool(name="const", bufs=1))
            mid_stack = ExitStack()
            ppsum = mid_stack.enter_context(
                tc.tile_pool(name="ppsum", bufs=1, space="PSUM")
            )
            gpsum = mid_stack.enter_context(
                tc.tile_pool(name="gpsum", bufs=1, space="PSUM")
            )
            rpsum = mid_stack.enter_context(
                tc.tile_pool(name="rpsum", bufs=1, space="PSUM")
            )
            wpool = stack.enter_context(tc.tile_pool(name="wqk", bufs=3))
            qkpool = stack.enter_context(tc.tile_pool(name="qk", bufs=2))
            exgp = stack.enter_context(tc.tile_pool(name="exg", bufs=18))
            exrp = stack.enter_context(tc.tile_pool(name="exr", bufs=10))
            zp = stack.enter_context(tc.tile_pool(name="z", bufs=2))
            bcp = stack.enter_context(tc.tile_pool(name="bc", bufs=2))
            rawp = stack.enter_context(tc.tile_pool(name="raw", bufs=2))


            hsT_sb = const.tile([128, EO, T], BF16, tag="hsT")
            hsT_r = hsT.rearrange("(eo ep) t -> ep eo t", ep=128)
            v_sb = const.tile([128, SO, HP, VB], BF16, tag="v")
            v8_sb = const.tile([128, SO, HP, VB], F8, tag="v8")
            U16 = mybir.dt.uint16
            nc.vector.memset(v_sb[:, :, :, D : 2 * D].bitcast(mybir.dt.uint32), 0)
            nc.vector.memset(v_sb[:, :, :, D].bitcast(U16), 0x3F80)
            nc.vector.memset(v_sb[:, :, :, 96].bitcast(U16), 0x3F80)
            comb = const.tile([128, EO, T], BF16, tag="comb")
            bqk_sb = const.tile([128, 2 * H], F32, tag="bqk")
            bo_sb = const.tile([128, OO], F32, tag="bo")
            qr8_t = const.tile([128, 2, T], F8, tag="qr8")
            kr8_t = const.tile([128, 2, 2, T], F8, tag="kr8")
            nc.vector.memset(kr8_t[:, :, 1, :].bitcast(mybir.dt.uint32), 0)

            # head-0 weights + bias first on the SP queue; hsT split across
            # the SP and ACT queues (Pool's software DGE costs ~1us/DMA on
            # the Pool engine itself -- reserve it for non-urgent weights)
            nc.sync.dma_start(bqk_sb[:], bqk[:])

            def emit_hsT():
                for eo in range(4):
                    nc.sync.dma_start(hsT_sb[:, eo], hsT_r[:, eo])
                for eo in range(4, EO):
                    nc.scalar.dma_start(hsT_sb[:, eo], hsT_r[:, eo])

            def proj_steps(h, which):
                """Emission steps for combined Q or K proj of head h."""
                wt = wpool.tile([128, EO, 128], BF16, tag="w")
                srcw = wq if which == "q" else wk
                wq_queue = nc.scalar if h == 0 else nc.gpsimd
                wq_queue.dma_start(
                    wt[:], srcw[:, h, :].rearrange("(eo ep) m -> ep eo m", ep=128)
                )
                dst = qkpool.tile([64, T], BF16, tag=which)
                hb = h % 2
                bias_col = 2 * h if which == "q" else 2 * h + 1
                state = {}

                pool_, ptag = (
                    (gpsum, "sg") if (h == 0 and which == "k") else (ppsum, "proj")
                )

                def mk_mm(nh, eo):
                    def step():
                        if nh == 0 and eo == 0:
                            state[0] = pool_.tile([128, T], F32, tag=ptag,
                                                  name=f"p{which}{h}")
                        nc.tensor.matmul(
                            state[0][:, nh * 512 : (nh + 1) * 512],
                            wt[:, eo, :],
                            hsT_sb[:, eo, nh * 512 : (nh + 1) * 512],
                            start=(eo == 0),
                            stop=(eo == EO - 1),
                        )
                        if eo == EO - 1:
                            sl = slice(nh * 512, (nh + 1) * 512)
                            nc.vector.tensor_scalar_add(
                                dst[:, sl],
                                state[0][0:64, sl],
                                bqk_sb[0:64, bias_col : bias_col + 1],
                            )
                            if which == "q":
                                r8dst = qr8_t[64:128, hb, sl]
                            else:
                                r8dst = kr8_t[64:128, hb, 0, sl]
                            # rdr bias rows arrive pre-scaled by SW from the
                            # host, so the drain is in*SW + bias16. Heads 0/1
                            # use the scalar engine (idle during the start).
                            if h <= 1:
                                nc.scalar.activation(
                                    r8dst,
                                    state[0][64:128, sl],
                                    mybir.ActivationFunctionType.Identity,
                                    bias=bqk_sb[64:128, bias_col : bias_col + 1],
                                    scale=SW,
                                )
                            else:
                                nc.vector.tensor_scalar(
                                    r8dst,
                                    state[0][64:128, sl],
                                    SW,
                                    bqk_sb[64:128, bias_col : bias_col + 1],
                                    MULT,
                                    ADD,
                                )
                    return step

                return dst, [mk_mm(nh, eo) for nh in range(2) for eo in range(EO)]

            def scores_step(h, s, Qh, Kh, exg_tiles, exr_tiles):
                hb = h % 2
                ssl = slice(s * 128, (s + 1) * 128)
                # rdr first: its psum ring frees one ACT op earlier, so the
                # scalar engine never idles across head boundaries
                psr = rpsum.tile([128, T], F32, tag="sr")
                qr_mv = qr8_t[64:128, hb, :]
                for nh in range(2):
                    tsl = slice(nh * 512, (nh + 1) * 512)
                    nc.tensor.matmul(
                        psr[:, tsl],
                        kr8_t[64:128, hb, :, ssl],
                        qr_mv[:, None, tsl].broadcast_to((64, 2, 512)),
                        start=True,
                        stop=True,
                        perf_mode=DRM,
                    )
                psg = gpsum.tile([128, T], F32, tag="sg")
                for nh in range(2):
                    nc.tensor.matmul(
                        psg[:, nh * 512 : (nh + 1) * 512],
                        Kh[:, ssl],
                        Qh[:, nh * 512 : (nh + 1) * 512],
                        start=True,
                        stop=True,
                    )
                if s % 2 == 0:
                    exr_tiles.append(exrp.tile([128, 2, T], F8, tag="exr", name=f"exr{h}_{s}"))
                nc.scalar.activation(
                    exr_tiles[s // 2][:, s % 2, :], psr[:], EXP,
                    scale=SCALING / (SW * SW),
                )
                exg = exgp.tile([128, T], BF16, tag="exg", name=f"exg{h}_{s}")
                nc.scalar.activation(exg[:], psg[:], EXP, scale=SCALING)
                exg_tiles.append(exg)

            def attn_steps(h, exg_tiles, exr_tiles, apsum):
                """Attention + normalize for head h as a list of emission
                steps (matmul chunks + tail), so it can be pumped into the
                next head's scores loop."""
                par_odd = h % 2
                abase = 64 * par_odd
                zrow = 64 if par_odd == 0 else 32
                voff = 64 * par_odd
                hp = h // 2
                steps = []
                state = {}

                def mk_mms(th, grp):
                    def step():
                        tsl = slice(th * 512, (th + 1) * 512)
                        if grp == 0:
                            state[th] = (
                                apsum.tile([128, 512], F32, tag="ag",
                                           name=f"pag{h}_{th}"),
                                apsum.tile([128, 512], F32, tag="ar",
                                           name=f"par{h}_{th}"),
                            )
                        pag, par_ = state[th]
                        if grp < 2:
                            for s in range(4 * grp, 4 * grp + 4):
                                nc.tensor.matmul(
                                    pag[:],
                                    v_sb[:, s, hp, voff : voff + 128],
                                    exg_tiles[s][:, tsl],
                                    start=(s == 0),
                                    stop=(s == SO - 1),
                                )
                        else:
                            for i in range(SO // 2):
                                nc.tensor.matmul(
                                    par_[:],
                                    v8_sb[:, 2 * i : 2 * i + 2, hp,
                                          voff : voff + 128],
                                    exr_tiles[i][:, :, tsl],
                                    start=(i == 0),
                                    stop=(i == SO // 2 - 1),
                                    perf_mode=DRM,
                                )
                    return step

                def mk_tail(th):
                    def step():
                        tsl = slice(th * 512, (th + 1) * 512)
                        pag, par_ = state[th]
                        zz = zp.tile([128, 2 * 512], F32, tag="zz",
                                     name=f"zz{h}_{th}")
                        zsl = slice(zrow, zrow + 1)
                        nc.vector.reciprocal(zz[zsl, 0:512], pag[zsl, :])
                        nc.vector.reciprocal(zz[zsl, 512:1024], par_[zsl, :])
                        nc.sync.dma_start(zz[0:1, 0:512], zz[zsl, 0:512])
                        nc.sync.dma_start(zz[0:1, 512:1024], zz[zsl, 512:1024])
                        bcg = bcp.tile([128, 512], F32, tag="bg",
                                       name=f"bg{h}_{th}")
                        bcr = bcp.tile([128, 512], F32, tag="br",
                                       name=f"br{h}_{th}")
                        nc.gpsimd.partition_broadcast(bcg[:], zz[0:1, 0:512])
                        nc.gpsimd.partition_broadcast(bcr[:], zz[0:1, 512:1024])
                        asl = slice(abase, abase + 64)
                        rawg = rawp.tile([128, 512], F32, tag="rg",
                                         name=f"rg{h}_{th}")
                        rawr = rawp.tile([128, 512], F32, tag="rr",
                                         name=f"rr{h}_{th}")
                        nc.vector.scalar_tensor_tensor(
                            rawg[asl, :], pag[asl, :], W_G, bcg[asl, :],
                            MULT, MULT,
                        )
                        nc.vector.scalar_tensor_tensor(
                            rawr[asl, :], par_[asl, :], W_R, bcr[asl, :],
                            MULT, MULT,
                        )
                        nc.vector.tensor_add(
                            comb[asl, hp, tsl], rawg[asl, :], rawr[asl, :]
                        )
                    return step

                for th in range(2):
                    steps += [mk_mms(th, 0), mk_mms(th, 1), mk_mms(th, 2),
                              mk_tail(th)]
                return steps

            # ---- proj of head 0, emitted directly (DMA-paced) --------------
            Qh, steps_q0 = proj_steps(0, "q")
            Kh, steps_k0 = proj_steps(0, "k")
            emit_hsT()
            for st in steps_q0 + steps_k0:
                st()

            # ---- start phase: V projection interleaved with head-0 scores --
            exg_prev = []
            exr_prev = []
            with tc.tile_pool(name="wvp", bufs=1) as wvp, tc.tile_pool(
                name="vpsum", bufs=1, space="PSUM"
            ) as vpsum:
                wv_sb = wvp.tile([128, EO, E], BF16, tag="wv")
                wv_r = wv.rearrange("(eo ep) o -> ep eo o", ep=128)
                # dummy Pool read of the last hsT chunk: holds the wv DMA
                # issues back so they don't interleave with hsT on the DMA
                # channel and delay projection 0
                wvgate = wvp.tile([1, 2], BF16, tag="wvgate")
                nc.gpsimd.tensor_scalar(
                    wvgate[0:1, :], hsT_sb[0:1, EO - 1, 0:2], 1.0, None, MULT
                )
                for eo in range(EO):
                    nc.gpsimd.dma_start(wv_sb[:, eo], wv_r[:, eo])
                nc.gpsimd.dma_start(bo_sb[:], bo[:])
                nextQ, steps_q1 = proj_steps(1, "q")
                nextK, steps_k1 = proj_steps(1, "k")
                pump = steps_q1 + steps_k1
                for so in range(SO):
                    scores_step(0, so, Qh, Kh, exg_prev, exr_prev)
                    pv = vpsum.tile([128, T], F32, tag="vproj", name=f"pv{so}")
                    for eo in range(EO):
                        for nh in range(2):
                            nc.tensor.matmul(
                                pv[:, nh * 512 : (nh + 1) * 512],
                                hsT_sb[:, eo, so * 128 : (so + 1) * 128],
                                wv_sb[:, eo, nh * 512 : (nh + 1) * 512],
                                start=(eo == 0),
                                stop=(eo == EO - 1),
                            )
                    for _ in range(4):
                        if pump:
                            pump.pop(0)()
                    pv4 = pv.rearrange("p (m two d) -> p m two d", two=2, d=D)
                    nc.vector.tensor_copy(v_sb[:, so, :, 0:D], pv4[:, :, 0, :])
                    nc.vector.tensor_copy(
                        v_sb[:, so, :, 128 : 128 + D], pv4[:, :, 1, :]
                    )
                    nc.gpsimd.tensor_scalar(
                        v8_sb[:, so], v_sb[:, so], 1.0, None, MULT
                    )
                while pump:
                    pump.pop(0)()
            Qh, Kh = nextQ, nextK

            tailp = stack.enter_context(tc.tile_pool(name="tail", bufs=8))
            outp2 = stack.enter_context(tc.tile_pool(name="opart", bufs=6))

            rings = [(ppsum, "proj"), (gpsum, "sg"), (rpsum, "sr")]
            NPRE = 6
            opart = {}
            po_pre = {}

            def pre_chunk(j, elo, ehi):
                def step():
                    if elo == 0:
                        pool_, ptag = rings[j % 3]
                        po_pre[j] = pool_.tile(
                            [128, T], F32, tag=ptag, name=f"poA{j}"
                        )
                    for nh in range(2):
                        for eo in range(elo, ehi):
                            nc.tensor.matmul(
                                po_pre[j][:, nh * 512 : (nh + 1) * 512],
                                wo_tiles[j][:, eo, :],
                                comb[:, eo, nh * 512 : (nh + 1) * 512],
                                start=(eo == 0),
                                stop=(eo == ehi - 1 and ehi == 7),
                            )
                    if ehi == 7:
                        op = outp2.tile([128, T], BF16, tag="opart",
                                        name=f"opart{j}")
                        opart[j] = op
                        nc.vector.tensor_copy(op[:], po_pre[j][:])
                return step

            # ppsum-ring js (0,3,6) can prework during head 15's scores
            # (no proj(16) uses that ring); gpsum/rpsum js go in the flush.
            fill_pp = []
            fill = [pre_chunk(j, elo, ehi) for j in range(NPRE)
                    for (elo, ehi) in ((0, 4), (4, 7))]

            # ---- heads: scores(h) first, attention(h-1) + proj(h+1) pumped --
            apsum = mid_stack.enter_context(
                tc.tile_pool(name="apsum", bufs=1, space="PSUM")
            )
            wo_tiles = []
            for h in range(1, H + 1):
                exg_cur = []
                exr_cur = []
                atn = attn_steps(h - 1, exg_prev, exr_prev, apsum)
                if h < H:
                    pump = []
                    if h + 1 < H:
                        nextQ, steps_q = proj_steps(h + 1, "q")
                        nextK, steps_k = proj_steps(h + 1, "k")
                        pump = steps_q + steps_k
                    if h == H - 3:
                        wo_r = wo.rearrange(
                            "(eo ep) (oo m) -> oo ep eo m", ep=128, m=128
                        )
                        for j in range(OO):
                            wt = tailp.tile([128, EO, 128], BF16, tag="wo",
                                            name=f"wo{j}")
                            nc.gpsimd.dma_start(wt[:], wo_r[j])
                            wo_tiles.append(wt)
                    for s in range(SO):
                        scores_step(h, s, Qh, Kh, exg_cur, exr_cur)
                        if atn:
                            atn.pop(0)()
                        for _ in range(4):
                            if pump:
                                pump.pop(0)()
                            elif h == H - 1 and fill_pp and s >= 2:
                                fill_pp.pop(0)()
                    while pump:
                        pump.pop(0)()
                if h == H:
                    fill = fill_pp + fill
                    while atn:
                        atn.pop(0)()
                        if fill:
                            fill.pop(0)()
                    while fill:
                        fill.pop(0)()
                else:
                    while atn:
                        atn.pop(0)()
                exg_prev, exr_prev = exg_cur, exr_cur
                if h < H - 1:
                    Qh, Kh = nextQ, nextK

            # ---------------- output projection -----------------------------
            with tc.tile_pool(name="outsb", bufs=2) as outp:
                for j in range(OO):
                    wt = wo_tiles[j]
                    pool_, ptag = [(ppsum, "proj"), (gpsum, "sg"),
                                   (rpsum, "sr")][j % 3]
                    po = pool_.tile([128, T], F32, tag=ptag, name=f"po{j}")
                    ot = outp.tile([128, T], F32, tag="ot", name=f"ot{j}")
                    pre = j < NPRE
                    for nh in range(2):
                        sl = slice(nh * 512, (nh + 1) * 512)
                        for eo in range(7 if pre else 0, EO):
                            nc.tensor.matmul(
                                po[:, sl],
                                wt[:, eo, :],
                                comb[:, eo, sl],
                                start=(eo == 7 if pre else eo == 0),
                                stop=(eo == EO - 1),
                            )
                        if pre:
                            nc.vector.scalar_tensor_tensor(
                                ot[:, sl], po[:, sl],
                                bo_sb[:, j : j + 1], opart[j][:, sl],
                                ADD, ADD,
                            )
                        else:
                            nc.vector.tensor_scalar_add(
                                ot[:, sl], po[:, sl], bo_sb[:, j : j + 1]
                            )
                        nc.sync.dma_start(
                            outT[j * 128 : (j + 1) * 128, sl], ot[:, sl]
                        )

            mid_stack.close()

    nc.finalize()
    return nc


_NC_CACHE = {}


def get_nc():
    if "nc" not in _NC_CACHE:
        _NC_CACHE["nc"] = build_nc()
    return _NC_CACHE["nc"]


def _host_prep(hidden_states, reader_token, Wq, bq, Wk, bk, Wv, bv, Wo, bo,
               RWq, Rbq, RWk, Rbk, RWv, Rbv):
    f = np.float32
    hs = np.asarray(hidden_states, f)
    tok = np.asarray(reader_token).astype(np.int64)
    WqT = np.ascontiguousarray(np.asarray(Wq, f).T)
    WkT = np.ascontiguousarray(np.asarray(Wk, f).T)
    WvT = np.ascontiguousarray(np.asarray(Wv, f).T).astype(NPBF)
    WoT = np.ascontiguousarray(np.asarray(Wo, f).T).astype(NPBF)
    RWqT = np.transpose(np.asarray(RWq, f), (0, 2, 1))
    RWkT = np.transpose(np.asarray(RWk, f), (0, 2, 1))
    bq = np.asarray(bq, f); bk = np.asarray(bk, f)
    bv = np.asarray(bv, f); bo_ = np.asarray(bo, f)
    Rbq = np.asarray(Rbq, f); Rbk = np.asarray(Rbk, f)

    bo_eff = bo_ + 0.5 * (np.asarray(Wo, f) @ bv)
    bo_t = np.ascontiguousarray(bo_eff.reshape(OO, 128).T)

    WqT_h = WqT.reshape(E, H, D)
    WkT_h = WkT.reshape(E, H, D)

    in_maps = []
    percore = {}
    for b in range(B):
        g = int(tok[b])
        if g not in percore:
            wqc = np.empty((E, H, 128), f)
            wqc[:, :, :D] = WqT_h
            wqc[:, :, D:] = RWqT[g].reshape(E, H, D)
            wkc = np.empty((E, H, 128), f)
            wkc[:, :, :D] = WkT_h
            wkc[:, :, D:] = RWkT[g].reshape(E, H, D)
            bqk_t = np.empty((128, 2 * H), f)
            bqk_t[:D, 0::2] = bq.reshape(H, D).T
            bqk_t[D:, 0::2] = SW * Rbq[g].reshape(H, D).T
            bqk_t[:D, 1::2] = bk.reshape(H, D).T
            bqk_t[D:, 1::2] = SW * Rbk[g].reshape(H, D).T
            percore[g] = (wqc.astype(NPBF), wkc.astype(NPBF), bqk_t)
        wqc, wkc, bqk_t = percore[g]
        in_maps.append(
            {
                "hsT": np.ascontiguousarray(hs[b].T).astype(NPBF),
                "wq": wqc,
                "wk": wkc,
                "wv": WvT,
                "wo": WoT,
                "bqk": bqk_t,
                "bo": bo_t,
            }
        )
    return in_maps


def kernel(**inputs) -> np.ndarray:
    in_maps = _host_prep(**inputs)
    nc = get_nc()
    res = run_bass_kernel_spmd(nc, in_maps, list(range(B)))
    out = np.stack([res.results[c]["outT"].T for c in range(B)], axis=0)
    return np.ascontiguousarray(out.astype(np.float32))
